# revision 71
# baseline (speedup 1.0000x reference)
"""Multi-head attention (B=4, S=2048, D=1024, H=16) on 8 TRN2 NeuronCores.

Sharding: pure tensor-parallel over heads. Core c owns heads (2c, 2c+1) of
EVERY batch — i.e. columns [128c, 128(c+1)) of Wq/Wk/Wv and the matching
128 rows of Wo. Attention work per (batch, head) scales with
nk_b = ceil(valid_len_b / 128) key tiles, so giving each core 2 heads of
every batch balances the per-core load exactly no matter how skewed
valid_lens are. Row-parallel Wo produces per-core partial outputs
[B*S, D] (bf16); the host sums the 8 partials.

Per-core dataflow, per batch b (kt counts specialized at build time):
  QT_b[d',q] = (Xq_b Wq_c)^T   bf16, head dims on partitions (h0: 0:64,
                               h1: 64:128)
  KT_b[d',k] = (Xk_b Wk_c)^T   only the first nk_b*128 key positions
  V_b[k, h, 65]                (Xv_b Wv_c) + per-head ones column (softmax
                               denominator rides along row 64 of av)
  per 512-wide q chunk:
    per kt: scoresT[k,q] for both heads (PE row groups 0/64 -> concurrent)
            ex = exp(scores * scale + maskbias)   one ACT instr, both heads
            av_h[65, q] += V_h^T-ish @ ex_h       (PSUM accumulate over kt)
    1/denom = exp(-ln(denom)) on ACT (reads PSUM directly: never blocks the
      ACT FIFO on DVE), broadcast to 128 partitions via a ones-column PE
      matmul, normalize on DVE, h1 lands in OT[64:128] via SBUF->SBUF DMA
      (engines cannot shift partition bases; DMA can)
  out_b = OT^T-slices @ Wo_c   full 128-dim contraction

The emission order software-pipelines ACROSS batches: projection (A/B)
units of batch b+1 and the output projection (D) of batch b are
interleaved between the attention q-chunks, so the PE stream never sits
in a low-duty phase and the HAM clock gate stays at 8/8.

Masking is pure data: mb/ms [128, sum nk_b] hold per-key exp bias/scale;
valid_len==0 gives scale=0,bias=0 -> uniform attention over all S keys,
matching jax.nn.softmax of an all-masked row.
"""

import math

import numpy as np

B, S, D, H = 4, 2048, 1024, 16
HD = D // H  # 64
NCORES = 8
NEG = -1.0e6
P = 128

_PROG_CACHE = {}


def _patch_tile_drain():
    """The walrus build in this container rejects sem waits attached to the
    Tile end-of-kernel Drain ("Too many sync wait commands" / SIGABRT).
    Replace them with standalone EventSemaphore waits, which it accepts."""
    import concourse.tile as tile
    from concourse.vector_clock import ScopedClock

    if getattr(tile.TileContext, "_drain_patched", False):
        return

    def _drain_and_barrier(self, tick_clock, wait_clock):
        nc = self.nc
        drain_inst = nc.sync.drain()
        wait_clock.add_sem_waits(
            drain_inst.ins, ScopedClock({None: tick_clock.global_clock})
        )
        si = drain_inst.ins.sync_info
        waits = list(si.on_wait) if si is not None and si.on_wait else []
        if waits:
            si.on_wait.clear()
            by_id, by_name = {}, {}
            for h in wait_clock.sems.allocated().values():
                by_id[getattr(h, "id", None)] = h
                by_name[getattr(h, "name", None)] = h
            for w in waits:
                h = by_id.get(w.id) or by_name.get(w.ant_name)
                assert h is not None, f"no handle for sem {w.ant_name} ({w.id})"
                nc.sync.wait_ge(h, w.wait_value)
        nc.all_engine_barrier()
        assert self.sems is not None
        popped = nc._tile_sem_poison_stack.pop()
        assert popped is self._sem_poison
        nc.clear_and_free_semaphores(list(self.sems.allocated().values()))
        nc.all_engine_barrier()

    tile.TileContext._drain_and_barrier = _drain_and_barrier
    tile.TileContext._drain_patched = True


def _split_multi_waits(nc, mybir):
    """This container's walrus rejects instructions carrying more than one
    semaphore wait ("Too many sync wait commands"). Hoist excess waits into
    standalone EventSemaphore instructions on the same engine, inserted
    immediately before the instruction — same-engine stream order preserves
    the semantics exactly."""
    n_ev = 0
    for fn in nc.m.functions:
        for bb in fn.blocks:
            insts = bb.instructions
            out = []
            for inst in insts:
                si = inst.sync_info
                waits = list(si.on_wait) if si is not None and si.on_wait else []
                keep = 0 if inst.opcode == "Drain" else 1
                if len(waits) > keep:
                    excess = waits[: len(waits) - keep]
                    kept = waits[len(waits) - keep:]
                    si.on_wait.clear()
                    si.on_wait.extend(kept)
                    for w in excess:
                        ev = mybir.InstEventSemaphore(
                            name=f"{inst.name}-hw{n_ev}",
                            engine=inst.engine,
                        )
                        ev.sync_info = mybir.SyncInfo(on_wait=[w], on_update=[])
                        out.append(ev)
                        n_ev += 1
                out.append(inst)
            if n_ev:
                insts[:] = out
    return n_ev


def _build_program(nks: tuple):
    import concourse.bass as bass
    import concourse.mybir as mybir
    import concourse.tile as tile

    _patch_tile_drain()

    f32 = mybir.dt.float32
    bf16 = mybir.dt.bfloat16
    AF = mybir.ActivationFunctionType

    KT_tot = sum(nks)
    SK = KT_tot * P
    offs = [sum(nks[:b]) for b in range(B)]

    nc = bass.Bass()

    xq_d = nc.dram_tensor("xq", [B, D, S], bf16, kind="ExternalInput")
    xk_d = nc.dram_tensor("xk", [D, SK], bf16, kind="ExternalInput")
    xv_d = nc.dram_tensor("xv", [D, SK], bf16, kind="ExternalInput")
    wq_d = nc.dram_tensor("wq", [D, P], bf16, kind="ExternalInput")
    wk_d = nc.dram_tensor("wk", [D, P], bf16, kind="ExternalInput")
    wv_d = nc.dram_tensor("wv", [D, P], bf16, kind="ExternalInput")
    wo_d = nc.dram_tensor("wo", [P, D], bf16, kind="ExternalInput")
    mb_d = nc.dram_tensor("mb", [P, KT_tot], f32, kind="ExternalInput")
    ms_d = nc.dram_tensor("ms", [P, KT_tot], f32, kind="ExternalInput")
    out_d = nc.dram_tensor("out", [B * S, D], bf16, kind="ExternalOutput")

    with tile.TileContext(nc) as tc:
        with (
            tc.tile_pool(name="pp", bufs=1) as pp,
            tc.tile_pool(name="qtp", bufs=2) as qtp,
            tc.tile_pool(name="ktp", bufs=2) as ktp,
            tc.tile_pool(name="vp", bufs=2) as vp,
            tc.tile_pool(name="otp", bufs=2) as otp,
            tc.tile_pool(name="t1p", bufs=3) as t1p,
            tc.tile_pool(name="xtp", bufs=4) as xtp,
            tc.tile_pool(name="expp", bufs=4) as expp,
            tc.tile_pool(name="rcpp", bufs=2) as rcpp,
            tc.tile_pool(name="bcsp", bufs=2) as bcsp,
            tc.tile_pool(name="outp", bufs=3) as outp,
            tc.tile_pool(name="psA", bufs=3, space="PSUM") as psA,
            tc.tile_pool(name="psB", bufs=1, space="PSUM") as psB,
        ):
            # persistent: weights, masks, ones row
            wq = pp.tile([P, 8, P], bf16, name="wq")
            wk = pp.tile([P, 8, P], bf16, name="wk")
            wv = pp.tile([P, 8, P], bf16, name="wv")
            wo = pp.tile([P, D], bf16, name="wo")
            mb = pp.tile([P, KT_tot], f32, name="mb")
            msc = pp.tile([P, KT_tot], f32, name="msc")
            # ones row lives at partition 64 so it can pair with the
            # denominator row of avb (PE requires lhsT/rhs at the same base)
            ones65 = pp.tile([HD + 1, P], bf16, name="ones65")

            nc.sync.dma_start(wq[:], wq_d[:, :].rearrange("(a p) c -> p a c", p=P))
            nc.sync.dma_start(wk[:], wk_d[:, :].rearrange("(a p) c -> p a c", p=P))
            nc.sync.dma_start(wv[:], wv_d[:, :].rearrange("(a p) c -> p a c", p=P))
            nc.any.memset(ones65[:], 1.0)

            # warm-up matmuls on throwaway data: the HAM clock gate needs
            # ~3.4us of sustained PE activity to lift the 1.2GHz cold
            # throttle, and the first real matmul waits ~15us for input
            # DMA anyway. Results land in a PSUM tile nothing reads.
            wrm = pp.tile([P, 512], bf16, name="wrm")
            nc.vector.memset(wrm[:], 0.0)
            for i in range(36):
                pw = psA.tile([P, 2, 512], f32, name="pw", tag="A")
                nc.tensor.matmul(
                    pw[:, 0, :], lhsT=wrm[:, 0:P], rhs=wrm[:],
                    start=True, stop=True,
                )

            # per-batch persistent tiles, allocated lazily by the units
            tiles = {}

            def get_tiles(b):
                if b not in tiles:
                    nk = nks[b]
                    tiles[b] = dict(
                        QT=qtp.tile([P, S], bf16, name=f"QT{b}", tag="qt"),
                        KT=ktp.tile([P, nk * P], bf16, name=f"KT{b}", tag="kt"),
                        V=vp.tile(
                            [P, nk, 2, HD + 1], bf16, name=f"V{b}", tag="v"
                        ),
                        OT=otp.tile([P, S], bf16, name=f"OT{b}", tag="ot"),
                    )
                return tiles[b]

            def units_AB(b):
                """Projection work units for batch b: QT/KT chunk-pairs with
                V key tiles spliced in."""
                nk = nks[b]
                off = offs[b]
                skb = nk * P
                t = get_tiles(b)
                xq_re = xq_d[b].rearrange("(a p) s -> p a s", p=P)
                xk_re = xk_d[:, off * P:off * P + skb].rearrange(
                    "(a p) s -> p a s", p=P
                )
                xv_re = xv_d[:, off * P:off * P + skb].rearrange(
                    "(a p) s -> p a s", p=P
                )

                def proj_pair(w_sb, x_re, dst, chunks):
                    def emit():
                        base = chunks[0][0]
                        wt = sum(w for _, w in chunks)
                        # one DMA for the pair: 2KB contiguous DRAM lines
                        xs = xtp.tile([P, 8, 1024], bf16, name="xs", tag="xt")
                        nc.sync.dma_start(
                            xs[:, :, 0:wt], x_re[:, :, base:base + wt]
                        )
                        pjs = [
                            psA.tile([P, 2, 512], f32, name="pj", tag="A")
                            for _ in chunks
                        ]
                        for a in range(8):
                            for j, (s0, w) in enumerate(chunks):
                                nc.tensor.matmul(
                                    pjs[j][:, 0, 0:w],
                                    lhsT=w_sb[:, a, :],
                                    rhs=xs[:, a, s0 - base:s0 - base + w],
                                    start=(a == 0),
                                    stop=(a == 7),
                                )
                        for j, (s0, w) in enumerate(chunks):
                            nc.vector.tensor_copy(
                                out=dst[:, s0:s0 + w], in_=pjs[j][:, 0, 0:w]
                            )
                    return emit

                def v_group(kts):
                    def emit():
                        for kt in kts:
                            xvt = xtp.tile(
                                [P, 8, P], bf16, name="xvt", tag="xvt"
                            )
                            nc.sync.dma_start(
                                xvt[:], xv_re[:, :, kt * P:(kt + 1) * P]
                            )
                            pv = psA.tile([P, 2, 512], f32, name="pv", tag="A")
                            for a in range(8):
                                nc.tensor.matmul(
                                    pv[:, 0, 0:P],
                                    lhsT=xvt[:, a, :],
                                    rhs=wv[:, a, :],
                                    start=(a == 0),
                                    stop=(a == 7),
                                )
                            nc.vector.memset(t["V"][:, kt, :, HD:HD + 1], 1.0)
                            nc.vector.tensor_copy(
                                out=t["V"][:, kt, :, 0:HD],
                                in_=pv[:, 0, 0:P].rearrange(
                                    "p (h c) -> p h c", c=HD
                                ),
                            )
                    return emit

                qchunks = [(s0, 512) for s0 in range(0, S, 512)]
                kchunks = [
                    (s0, min(512, skb - s0)) for s0 in range(0, skb, 512)
                ]
                work = [
                    proj_pair(wq, xq_re, t["QT"], qchunks[0:2]),
                    proj_pair(wq, xq_re, t["QT"], qchunks[2:4]),
                ]
                work += [
                    proj_pair(wk, xk_re, t["KT"], kchunks[p0:p0 + 2])
                    for p0 in range(0, len(kchunks), 2)
                ]
                vkts = list(range(nk))
                nslots = len(work)
                vgroups = [vkts[i::nslots] for i in range(nslots)]
                out = []
                for w, vg in zip(work, vgroups):
                    out.append(w)
                    if vg:
                        out.append(v_group(vg))
                return out

            def unit_C(b, qh, prev_tail):
                """One attention q chunk for batch b. The PE-side normalize
                tail of the PREVIOUS chunk (bc broadcast matmuls + muls) is
                spliced in after this chunk's second kt iteration: the PE
                queue is strict FIFO, so emitting bc right after its own kt
                loop head-of-line-blocks the PE ~2.4us while ACT computes
                the reciprocal."""
                nk = nks[b]
                off = offs[b]
                t = get_tiles(b)

                def emit():
                    QT, KT, V, OT = t["QT"], t["KT"], t["V"], t["OT"]
                    q0 = qh * 512
                    avs = psB.tile(
                        [HD + 1, 2, 512], f32, name="avs", tag="av"
                    )
                    for kt in range(nk):
                        scs = psA.tile([P, 2, 512], f32, name="scs", tag="A")
                        for h in range(2):
                            pb = h * HD
                            nc.tensor.matmul(
                                scs[:, h, :],
                                lhsT=KT[pb:pb + HD, kt * P:(kt + 1) * P],
                                rhs=QT[pb:pb + HD, q0:q0 + 512],
                                start=True,
                                stop=True,
                            )
                        ex = expp.tile([P, 2, 512], bf16, name="ex", tag="ex")
                        # score scaling is folded into the host-side xk data;
                        # scale=1.0 stays an immediate (no per-instr AP read)
                        nc.scalar.activation(
                            ex[:],
                            scs[:],
                            AF.Exp,
                            bias=mb[:, off + kt:off + kt + 1],
                        )
                        for h in range(2):
                            nc.tensor.matmul(
                                avs[:, h, :],
                                lhsT=V[:, kt, h, :],
                                rhs=ex[:, h, :],
                                start=(kt == 0),
                                stop=(kt == nk - 1),
                            )
                        if kt == min(1, nk - 1) and prev_tail is not None:
                            prev_tail[0]()
                    # free the accumulator banks for the next q chunk asap,
                    # then scatter the 1024 denominators across all 128
                    # partitions by DMA (ACT cost is free-dim cycles per
                    # LANE: the scattered layout makes both reciprocal table
                    # passes ~0.17us instead of ~1.0us each)
                    avb = bcsp.tile(
                        [HD + 1, 2, 512], bf16, name="avb", tag="avb"
                    )
                    nc.vector.tensor_copy(out=avb[:], in_=avs[:])
                    rcT = rcpp.tile([P, 8], bf16, name="rcT", tag="rcT")
                    nc.sync.dma_start(
                        rcT[:],
                        avb[HD:HD + 1, :, :].rearrange("p a b -> p (a b)"),
                    )
                    if prev_tail is not None:
                        prev_tail[1]()

                    def tail_a():
                        # 1/denom = exp(-ln(.)) on the scattered tile; the
                        # scatter DMA has long finished by the time this runs
                        # (one kt-loop later), so the ACT FIFO never stalls
                        lnT = rcpp.tile([P, 8], f32, name="lnT", tag="lnT")
                        nc.scalar.activation(lnT[:], rcT[:], AF.Ln)
                        rcbT = rcpp.tile([P, 8], bf16, name="rcbT", tag="rcbT")
                        nc.scalar.activation(rcbT[:], lnT[:], AF.Exp,
                                             scale=-1.0)
                        rcb = rcpp.tile([1, 2, 512], bf16, name="rcb",
                                        tag="rcb")
                        nc.sync.dma_start(
                            rcb[:].rearrange("p a b -> p (a b)"), rcbT[:]
                        )
                        return rcb

                    state = {}

                    def tail_a_run():
                        state["rcb"] = tail_a()

                    def tail_b():
                        # broadcast the reciprocals and normalize; the muls
                        # read the broadcast straight from PSUM (no copy)
                        rcb = state["rcb"]
                        bc = psA.tile([P, 2, 512], f32, name="bc", tag="A")
                        for h in range(2):
                            nc.tensor.matmul(
                                bc[:, h, :],
                                lhsT=ones65[0:1, :],
                                rhs=rcb[:, h, :],
                                start=True,
                                stop=True,
                            )
                        nc.vector.tensor_mul(
                            out=OT[0:HD, q0:q0 + 512],
                            in0=avb[0:HD, 0, :],
                            in1=bc[0:HD, 0, :],
                        )
                        t1 = t1p.tile([HD, 512], bf16, name="t1", tag="t1")
                        nc.vector.tensor_mul(
                            out=t1[:],
                            in0=avb[0:HD, 1, :],
                            in1=bc[0:HD, 1, :],
                        )
                        nc.sync.dma_start(OT[HD:P, q0:q0 + 512], t1[:])
                    return (tail_a_run, tail_b)
                return emit

            def unit_D(b, qts):
                """Output projection for a group of q tiles of batch b."""
                t = get_tiles(b)

                def emit():
                    OT = t["OT"]
                    for qt in qts:
                        ob = outp.tile([P, D], bf16, name="ob", tag="ob")
                        wps = psA.tile([P, 2, 512], f32, name="wps", tag="A")
                        for ch2 in range(2):
                            nc.tensor.matmul(
                                wps[:, ch2, :],
                                lhsT=OT[:, qt * P:(qt + 1) * P],
                                rhs=wo[:, ch2 * 512:(ch2 + 1) * 512],
                                start=True,
                                stop=True,
                            )
                        for ch2 in range(2):
                            if qt % 2 == 0:
                                nc.vector.tensor_copy(
                                    out=ob[:, ch2 * 512:(ch2 + 1) * 512],
                                    in_=wps[:, ch2, :],
                                )
                            else:
                                nc.scalar.copy(
                                    out=ob[:, ch2 * 512:(ch2 + 1) * 512],
                                    in_=wps[:, ch2, :],
                                )
                        nc.sync.dma_start(
                            out_d[(b * 16 + qt) * P:(b * 16 + qt + 1) * P, :],
                            ob[:],
                        )
                return emit

            # ---- emission schedule: software pipeline across batches.
            # filler units (next batch's projections, current batch's output
            # projection) are spliced between attention q chunks.
            # lightest batch first (smallest input DMA exposure at the cold
            # start), heaviest second (its projections hide behind the first
            # attention), second-heaviest last so the final C region can
            # absorb the previous batch's deferred output-projection fillers
            bsrt = sorted(range(B), key=lambda bb: -nks[bb])
            border = [bsrt[3], bsrt[0]] + bsrt[2:3] + [bsrt[1]]
            first_units = units_AB(border[0])
            first_units[0]()
            # non-critical loads issued after the first projection's inputs
            nc.sync.dma_start(wo[:], wo_d[:, :])
            nc.sync.dma_start(mb[:], mb_d[:, :])
            nc.sync.dma_start(msc[:], ms_d[:, :])
            for u in first_units[1:]:
                u()
            pending_D = []
            tails = []
            for bi, b in enumerate(border):
                nxt = border[bi + 1] if bi + 1 < B else None
                fillers = list(pending_D)
                if nxt is not None:
                    fillers += units_AB(nxt)
                # spread fillers across the 4 q chunks (after each chunk)
                sched = [[] for _ in range(4)]
                for i, f in enumerate(fillers):
                    sched[i % 4].append(f)
                last = nxt is None
                for qh in range(4):
                    prev = tails.pop(0) if len(tails) >= 1 else None
                    tails.append(unit_C(b, qh, prev)())
                    for f in sched[qh]:
                        f()
                    if last and qh >= 2:
                        # the final batch has no successor C to host its D
                        # fillers; squeeze finished q chunks' D in here
                        unit_D(b, list(range((qh - 2) * 4, (qh - 2) * 4 + 4)))()
                pending_D = [unit_D(b, list(range(g * 4, g * 4 + 4)))
                             for g in (range(2, 4) if last else range(4))]
            for t in tails:
                t[0]()
                t[1]()
            for u in pending_D:
                u()

    _split_multi_waits(nc, mybir)
    return nc


def _get_program(nks: tuple):
    if nks not in _PROG_CACHE:
        _PROG_CACHE[nks] = _build_program(nks)
    return _PROG_CACHE[nks]


def kernel(**inputs) -> np.ndarray:
    import ml_dtypes
    from concourse.bass_utils import run_bass_kernel_spmd

    bf = ml_dtypes.bfloat16

    q = np.asarray(inputs["queries"], dtype=np.float32)
    k = np.asarray(inputs["keys"], dtype=np.float32)
    v = np.asarray(inputs["values"], dtype=np.float32)
    vl = np.asarray(inputs["valid_lens"]).astype(np.int64)
    Wq = np.asarray(inputs["Wq"], dtype=np.float32)
    Wk = np.asarray(inputs["Wk"], dtype=np.float32)
    Wv = np.asarray(inputs["Wv"], dtype=np.float32)
    Wo = np.asarray(inputs["Wo"], dtype=np.float32)

    nks = tuple(
        (S // P) if int(vl[b]) == 0
        else min(S // P, int(math.ceil(int(vl[b]) / P)))
        for b in range(B)
    )
    nc = _get_program(nks)

    KT_tot = sum(nks)
    offs = [sum(nks[:b]) for b in range(B)]

    # shared across cores (host arrays reused; staging per device is free).
    # The 1/sqrt(HD) score scale is folded into the keys (and a zero scale
    # for the valid_len==0 all-masked case -> exp(0)=1 uniform attention).
    xq = np.ascontiguousarray(q.transpose(0, 2, 1)).astype(bf)
    xk = np.concatenate(
        [
            k[b].T[:, : nks[b] * P]
            * (0.0 if int(vl[b]) == 0 else 1.0 / math.sqrt(HD))
            for b in range(B)
        ],
        axis=1,
    ).astype(bf)
    xv = np.concatenate(
        [v[b].T[:, : nks[b] * P] for b in range(B)], axis=1
    ).astype(bf)

    m_bias = np.empty((P, KT_tot), np.float32)
    m_scale = np.empty((P, KT_tot), np.float32)
    for b in range(B):
        vlb = int(vl[b])
        kk = (
            np.arange(nks[b])[None, :] * P + np.arange(P)[:, None]
        ).astype(np.int64)
        if vlb == 0:
            m_bias[:, offs[b]:offs[b] + nks[b]] = 0.0
            m_scale[:, offs[b]:offs[b] + nks[b]] = 0.0
        else:
            m_bias[:, offs[b]:offs[b] + nks[b]] = np.where(kk < vlb, 0.0, NEG)
            m_scale[:, offs[b]:offs[b] + nks[b]] = 1.0 / math.sqrt(HD)

    in_maps = []
    for c in range(NCORES):
        cols = slice(c * P, (c + 1) * P)
        in_maps.append(
            {
                "xq": xq,
                "xk": xk,
                "xv": xv,
                "wq": np.ascontiguousarray(Wq[:, cols]).astype(bf),
                "wk": np.ascontiguousarray(Wk[:, cols]).astype(bf),
                "wv": np.ascontiguousarray(Wv[:, cols]).astype(bf),
                "wo": np.ascontiguousarray(Wo[cols, :]).astype(bf),
                "mb": m_bias,
                "ms": m_scale,
            }
        )

    globals()["_LAST_IN_MAPS"] = in_maps
    res = run_bass_kernel_spmd(nc, in_maps, list(range(NCORES))).results

    acc = np.zeros((B * S, D), dtype=np.float32)
    for c in range(NCORES):
        acc += res[c]["out"].astype(np.float32)
    return acc.reshape(B, S, D)
